# revision 17
# baseline (speedup 1.0000x reference)
"""DynamicLoRAConv1d kernel for 8 Trainium2 NeuronCores.

Math: per-sample LoRA conv is linear in weights, so
  conv(x, W) + conv(x, dW_b) = conv(x, W + dW_b)
with dW_b = lora_scale * (B_b @ A_b).  The tiny per-sample effective weight
(W_eff = conv_w + dW_b, 32 x 128 x 64 x 5) is computed on host; the device
kernel does, per image (64ch x 2048):
  - conv1d(stride 2, pad 2, K=5) as 5 accumulated float32r matmuls per
    512-column half (rhs = stride-2 view of the padded x tile in SBUF)
  - bias + ReLU on ScalarE with accum_out giving per-channel sums
  - per-channel sum of y^2 via one DVE tensor_tensor_reduce per half
  - GroupNorm(4 groups): one 128x128 group-indicator matmul turns
    per-channel [S, Q] into per-channel broadcast [mean_g, E2_g]; then a few
    (128,1) ops produce per-channel scale/offset; final ScalarE pass applies
    y*scale+offset.
Sharding: data-parallel over Batch - core c gets samples 4c..4c+3
(= images 32c..32c+32).  No cross-core communication.
"""

import sys
from contextlib import ExitStack

import numpy as np

for _p in ("/opt/trn_rl_repo", "/opt/pypackages"):
    if _p not in sys.path:
        sys.path.append(_p)

import concourse.bacc as bacc
import concourse.bass as bass
import concourse.mybir as mybir
import concourse.tile as tile
from concourse.bass_utils import run_bass_kernel_spmd

F32 = mybir.dt.float32
F32R = mybir.dt.float32r
AF = mybir.ActivationFunctionType
ALU = mybir.AluOpType

N_CORES = 8
SAMPLES = 4      # samples per core
SENSORS = 8
IMGS = SAMPLES * SENSORS  # images per core
IN_C = 64
OUT_C = 128
KTAPS = 5
T = 2048
T_PAD = T + 4    # 2052
T_OUT = 1024
HALF = 512
EPS = 1e-5
G = 4
CPG = OUT_C // G  # channels per group

# Knobs for experiments (run_bass_kernel_spmd kwargs threaded by caller)
TRACE = False
LAST_RESULTS = None

_PROGRAM = None


def _build_program():
    nc = bacc.Bacc("TRN2", target_bir_lowering=False, debug=False)
    xin = nc.dram_tensor("xin", [IMGS, IN_C, T_PAD], F32R, kind="ExternalInput")
    wts = nc.dram_tensor("wts", [SAMPLES, IN_C, KTAPS * OUT_C], F32R,
                         kind="ExternalInput")
    cons = nc.dram_tensor("cons", [OUT_C, 4], F32, kind="ExternalInput")
    gmat = nc.dram_tensor("gmat", [OUT_C, OUT_C], F32, kind="ExternalInput")
    out = nc.dram_tensor("out", [IMGS, OUT_C, T_OUT], F32, kind="ExternalOutput")

    with ExitStack() as ctx:
        tc = ctx.enter_context(tile.TileContext(nc))
        cpool = ctx.enter_context(tc.tile_pool(name="cpool", bufs=1))
        xpool = ctx.enter_context(tc.tile_pool(name="xpool", bufs=4))
        ypool = ctx.enter_context(tc.tile_pool(name="ypool", bufs=3))
        opool = ctx.enter_context(tc.tile_pool(name="opool", bufs=3))
        spool = ctx.enter_context(tc.tile_pool(name="spool", bufs=4))
        pspool = ctx.enter_context(tc.tile_pool(name="pspool", bufs=2, space="PSUM"))
        stpool = ctx.enter_context(tc.tile_pool(name="stpool", bufs=2, space="PSUM"))

        # ---- persistent constants ----
        wt = cpool.tile([IN_C, SAMPLES * KTAPS * OUT_C], F32R)
        nc.sync.dma_start(
            out=wt[:].rearrange("p (s f) -> p s f", s=SAMPLES),
            in_=wts.ap().rearrange("s p f -> p s f"))
        ct = cpool.tile([OUT_C, 4], F32)
        nc.sync.dma_start(out=ct[:], in_=cons.ap()[:])
        gt = cpool.tile([OUT_C, OUT_C], F32)
        nc.sync.dma_start(out=gt[:], in_=gmat.ap()[:])
        bias_ap = ct[:, 0:1]
        gamma_ap = ct[:, 1:2]
        beta_ap = ct[:, 2:3]
        eps_ap = ct[:, 3:4]

        for i in range(IMGS):
            s = i // SENSORS
            # padded input image: xt[ci, j] = x_pad[ci, j], j = 2t + k
            xt = xpool.tile([IN_C, T_PAD], F32R, tag="xt")
            nc.sync.dma_start(out=xt[:], in_=xin.ap()[i])
            # even/odd view: xe[ci, u, par] = xt[ci, 2u + par]
            xe = xt[:].rearrange("p (u two) -> p u two", two=2)

            y = ypool.tile([OUT_C, T_OUT], F32, tag="y")
            bnraw = spool.tile([OUT_C, 12], F32, tag="bnraw")
            ps = [pspool.tile([OUT_C, HALF], F32, tag=f"ps{h}", name=f"ps{h}_{i}")
                  for h in range(2)]

            # conv: out[co, t] = sum_{k, ci} W[co,ci,k] * xt[ci, 2t+k]
            for k in range(KTAPS):
                w_ap = wt[:, (s * KTAPS + k) * OUT_C:(s * KTAPS + k + 1) * OUT_C]
                for h in range(2):
                    u0 = (k // 2) + h * HALF
                    rhs = xe[:, u0:u0 + HALF, (k % 2):(k % 2) + 1]
                    nc.tensor.matmul(ps[h][:], w_ap, rhs,
                                     start=(k == 0), stop=(k == KTAPS - 1))

            # bias + relu, then per-channel mean/var via bn_stats
            for h in range(2):
                yh = y[:, h * HALF:(h + 1) * HALF]
                nc.scalar.activation(yh, ps[h][:], AF.Relu,
                                     bias=bias_ap, scale=1.0)
                nc.vector.bn_stats(bnraw[:, 6 * h:6 * h + 6], yh)

            # sq2 = [mean_p, E2_p]  (E2 = var + mean^2)
            sq2 = spool.tile([OUT_C, 2], F32, tag="sq2")
            tmp0 = spool.tile([OUT_C, 1], F32, tag="tmp0")
            nc.vector.bn_aggr(sq2[:], bnraw[:])
            nc.vector.tensor_mul(tmp0[:], sq2[:, 0:1], sq2[:, 0:1])
            nc.vector.tensor_add(sq2[:, 1:2], sq2[:, 1:2], tmp0[:])

            # group means broadcast back to channels: st = G^T @ [mean, E2]
            st = stpool.tile([OUT_C, 2], F32, tag="st")
            nc.tensor.matmul(st[:], gt[:], sq2[:], start=True, stop=True)

            # per-channel scale/offset from group stats
            # (DVE may read at most one PSUM operand -> copy stats to SBUF)
            stat = spool.tile([OUT_C, 7], F32, tag="stat")
            sg = stat[:, 0:2]
            nc.vector.tensor_copy(sg, st[:])
            mean = stat[:, 0:1]
            e2 = stat[:, 1:2]
            m2, var, std, rstd, tmp = (stat[:, j:j + 1] for j in range(2, 7))
            nc.vector.tensor_mul(m2, mean, mean)
            nc.vector.tensor_sub(var, e2, m2)
            nc.scalar.activation(std, var, AF.Sqrt, bias=eps_ap)
            nc.vector.reciprocal(rstd, std)
            so = spool.tile([OUT_C, 2], F32, tag="so")
            scl = so[:, 0:1]
            off = so[:, 1:2]
            nc.vector.tensor_mul(scl, rstd, gamma_ap)
            nc.vector.tensor_mul(tmp, mean, scl)
            nc.vector.tensor_sub(off, beta_ap, tmp)

            ot = opool.tile([OUT_C, T_OUT], F32, tag="ot")
            for h in range(2):
                nc.scalar.activation(ot[:, h * HALF:(h + 1) * HALF],
                                     y[:, h * HALF:(h + 1) * HALF],
                                     AF.Identity, bias=off, scale=scl)
            nc.sync.dma_start(out=out.ap()[i], in_=ot[:])
    nc.compile()
    return nc


def get_program():
    global _PROGRAM
    if _PROGRAM is None:
        _PROGRAM = _build_program()
    return _PROGRAM


def _host_prep(x, A_flat, B_flat, conv_w, conv_b, gamma, beta, num_sensors, r,
               lora_scale):
    x = np.asarray(x, dtype=np.float32)
    A_flat = np.asarray(A_flat, dtype=np.float32)
    B_flat = np.asarray(B_flat, dtype=np.float32)
    conv_w = np.asarray(conv_w, dtype=np.float32)
    conv_b = np.asarray(conv_b, dtype=np.float32)
    gamma = np.asarray(gamma, dtype=np.float32)
    beta = np.asarray(beta, dtype=np.float32)
    batch = A_flat.shape[0]
    out_c, in_c, k = conv_w.shape
    ns = int(num_sensors)
    rr = int(r)
    ls = float(lora_scale)
    assert (batch, out_c, in_c, k) == (32, OUT_C, IN_C, KTAPS)
    assert ns == SENSORS and x.shape == (batch * ns, in_c, T)

    # per-sample effective weight, transposed for the PE (lhsT layout)
    A = A_flat.reshape(batch, rr, in_c * k)
    Bm = B_flat.reshape(batch, out_c, rr)
    delta = np.einsum("bor,brm->bom", Bm, A) * ls
    W = conv_w.reshape(1, out_c, in_c * k) + delta            # (B, out_c, in_c*k)
    Wt = W.reshape(batch, out_c, in_c, k).transpose(0, 2, 3, 1)  # (B, ci, k, co)
    Wt = np.ascontiguousarray(Wt.reshape(batch, in_c, k * out_c), dtype=np.float32)

    x_pad = np.zeros((batch * ns, in_c, T_PAD), dtype=np.float32)
    x_pad[:, :, 2:2 + T] = x

    eps_col = np.full_like(conv_b, EPS)
    cons = np.ascontiguousarray(np.stack([conv_b, gamma, beta, eps_col], axis=1),
                                dtype=np.float32)
    gm = np.kron(np.eye(G, dtype=np.float32),
                 np.full((CPG, CPG), 1.0 / CPG, dtype=np.float32))
    gm = np.ascontiguousarray(gm, dtype=np.float32)

    in_maps = []
    for c in range(N_CORES):
        in_maps.append({
            "xin": np.ascontiguousarray(x_pad[c * IMGS:(c + 1) * IMGS]),
            "wts": np.ascontiguousarray(Wt[c * SAMPLES:(c + 1) * SAMPLES]),
            "cons": cons,
            "gmat": gm,
        })
    return in_maps


def kernel(x, A_flat, B_flat, conv_w, conv_b, gamma, beta, num_sensors, r,
           lora_scale):
    global LAST_RESULTS
    in_maps = _host_prep(x, A_flat, B_flat, conv_w, conv_b, gamma, beta,
                         num_sensors, r, lora_scale)
    nc = get_program()
    res = run_bass_kernel_spmd(nc, in_maps, core_ids=list(range(N_CORES)),
                               trace=TRACE)
    LAST_RESULTS = res
    return np.concatenate([res.results[c]["out"] for c in range(N_CORES)],
                          axis=0)
